# revision 8
# baseline (speedup 1.0000x reference)
"""CommNet (B=4096, A=50, DIN=128, H=256, DOUT=64, K=2) on 8 TRN2 NeuronCores.

Data-parallel over the batch axis: 512 examples (25600 agent-tokens) per core,
weights replicated. On-chip layout is feature-major ([feature, token]) so every
layer's contraction dim sits on SBUF partitions; the host pre-transposes each
x shard once (numpy) so no on-chip transposes are needed.

Per comm step the concat [h, c] @ W is split as h @ W_top + c @ W_bot with the
1/50 agent-mean folded into W_bot on the host. The per-example c @ W_bot result
(computed transposed, with c as the stationary matmul operand) is broadcast
back over agents by a third accumulating matmul against a constant 0/1
selector, so the whole comm step stays on the PE and lands in one PSUM tile.

Matmul operands are fp16 (1 cyc/row, fast weight load; PSUM accumulation is
fp32). tanh runs on ScalarE over three bank-aligned PSUM sub-tiles per
ACTIVATE (N=1200) to amortize the per-op ~352-cycle overhead. VectorE does the
per-example agent-sum reductions (fp16 2x mode) and the decoder bias-add;
GpSimd only drives the casting DMAs.
"""

import numpy as np

import concourse.bacc as bacc
import concourse.bass as bass
import concourse.tile as tile
from concourse import mybir
from concourse.bass_utils import run_bass_kernel_spmd

N_CORES = 8
B, A, DIN, H, DOUT, K = 4096, 50, 128, 256, 64, 2
BS = B // N_CORES          # examples per core
TOK = BS * A               # tokens per core
ST_EX = 64                 # examples per supertile
ST = ST_EX * A             # 3200 tokens per supertile
SUB_EX = 8                 # examples per matmul sub-tile
SUB = SUB_EX * A           # 400 tokens (PSUM bank limit: N <= 512 fp32 accum)
NSUB = ST // SUB           # 8
BANK = 512                 # fp32 elems per PSUM bank
RSUB = 1600                # tokens per DVE reduce op (32 examples)

F32 = mybir.dt.float32
F16 = mybir.dt.float16
Tanh = mybir.ActivationFunctionType.Tanh

# tanh batching: groups of matmul sub-tiles sharing one PSUM tile + ACTIVATE
TANH_GROUPS = [(0, 1, 2), (3, 4, 5), (6, 7)]


def build_nc(n_supertiles=BS // ST_EX):
    tok = n_supertiles * ST
    nc = bacc.Bacc(
        "TRN2",
        target_bir_lowering=False,
        debug=False,
        enable_asserts=True,
        num_devices=N_CORES,
    )
    xT = nc.dram_tensor("xT", [DIN, tok], F32, kind="ExternalInput")
    w_enc = nc.dram_tensor("w_enc", [DIN, H], F32, kind="ExternalInput")
    b_enc = nc.dram_tensor("b_enc", [128, 2], F32, kind="ExternalInput")
    w_top = nc.dram_tensor("w_top", [K, 2, 128, H], F32, kind="ExternalInput")
    w_bot = nc.dram_tensor("w_bot", [K, 2, 128, H], F32, kind="ExternalInput")
    b_h = nc.dram_tensor("b_h", [128, K * 2], F32, kind="ExternalInput")
    w_dec = nc.dram_tensor("w_dec", [2, 128, DOUT], F32, kind="ExternalInput")
    b_dec = nc.dram_tensor("b_dec", [DOUT, 1], F32, kind="ExternalInput")
    sel = nc.dram_tensor("sel", [ST_EX, ST], F32, kind="ExternalInput")
    y = nc.dram_tensor("y", [DOUT, tok], F32, kind="ExternalOutput")

    with tile.TileContext(nc) as tc:
        with (
            tc.tile_pool(name="wpool", bufs=1) as wpool,
            tc.tile_pool(name="xpool", bufs=2) as xpool,
            tc.tile_pool(name="hpool", bufs=2) as hpool,
            tc.tile_pool(name="opool", bufs=2) as opool,
            tc.tile_pool(name="cpool", bufs=2) as cpool,
            tc.tile_pool(name="psmm", bufs=2, space=bass.MemorySpace.PSUM) as psmm,
            tc.tile_pool(name="psdec", bufs=1, space=bass.MemorySpace.PSUM) as psdec,
            tc.tile_pool(name="pscw", bufs=1, space=bass.MemorySpace.PSUM) as pscw,
        ):
            # --- weights: casting DMAs (f32 -> fp16), resident for the run ---
            wenc_sb = wpool.tile([DIN, H], F16)
            nc.gpsimd.dma_start(wenc_sb[:], w_enc[:])
            benc_sb = wpool.tile([128, 2], F32)
            nc.sync.dma_start(benc_sb[:], b_enc[:])
            wtop_sb = wpool.tile([128, K * 2 * H], F16)
            wbot_sb = wpool.tile([128, K * 2 * H], F16)
            for k in range(K):
                for kc in range(2):
                    off = (k * 2 + kc) * H
                    nc.gpsimd.dma_start(wtop_sb[:, off : off + H], w_top[k, kc])
                    nc.gpsimd.dma_start(wbot_sb[:, off : off + H], w_bot[k, kc])
            bh_sb = wpool.tile([128, K * 2], F32)
            nc.sync.dma_start(bh_sb[:], b_h[:])
            wdec_sb = wpool.tile([128, 2 * DOUT], F16)
            for kc in range(2):
                nc.gpsimd.dma_start(wdec_sb[:, kc * DOUT : (kc + 1) * DOUT], w_dec[kc])
            bdec_sb = wpool.tile([DOUT, 1], F32)
            nc.sync.dma_start(bdec_sb[:], b_dec[:])
            sel_sb = wpool.tile([ST_EX, ST], F16)
            nc.gpsimd.dma_start(sel_sb[:], sel[:])

            for s in range(n_supertiles):
                t0 = s * ST
                xt = xpool.tile([DIN, ST], F16, tag="xt")
                nc.gpsimd.dma_start(xt[:], xT[:, t0 : t0 + ST])
                hA = [hpool.tile([128, ST], F16, tag=f"hA{m}", name=f"hA{m}_{s}") for m in range(2)]
                hB = [hpool.tile([128, ST], F16, tag=f"hB{m}", name=f"hB{m}_{s}") for m in range(2)]

                def psum_view(ps, g):
                    # [128, g, SUB] strided view of the first g banks
                    return ps[:].rearrange("p (g b) -> p g b", b=BANK)[:, 0:g, 0:SUB]

                def tanh_grouped(m, grp, mm_emit, hout, bias):
                    # per-sub-tile matmul groups land bank-aligned in one
                    # multi-bank PSUM tile; one strided tanh covers them all
                    g = len(grp)
                    ps = psmm.tile(
                        [128, 3 * BANK], F32, tag="ps", name=f"ps_{s}_{m}_{grp[0]}"
                    )
                    for j, n in enumerate(grp):
                        mm_emit(ps[:, j * BANK : j * BANK + SUB], n)
                    hv = hout[
                        :, grp[0] * SUB : (grp[-1] + 1) * SUB
                    ].rearrange("p (g b) -> p g b", b=SUB)
                    nc.scalar.activation(hv, psum_view(ps, g), Tanh, bias=bias)

                # encoder: h = tanh(W_enc.T @ xT + b_enc)
                for m in range(2):
                    def enc_mm(out_ap, n, m=m):
                        nc.tensor.matmul(
                            out_ap,
                            wenc_sb[:, m * 128 : (m + 1) * 128],
                            xt[:, n * SUB : (n + 1) * SUB],
                            start=True,
                            stop=True,
                        )
                    for grp in TANH_GROUPS:
                        tanh_grouped(m, grp, enc_mm, hA[m], benc_sb[:, m : m + 1])

                hcur, hnxt = hA, hB
                for k in range(K):
                    # per-example agent sum (1/50 pre-folded into W_bot)
                    c_t = cpool.tile([128, 2, ST_EX], F16, tag="c")
                    for kc in range(2):
                        for n in range(ST // RSUB):
                            seg = hcur[kc][
                                :, n * RSUB : (n + 1) * RSUB
                            ].rearrange("p (b a) -> p b a", a=A)
                            nex = RSUB // A
                            with nc.allow_low_precision(
                                reason="fp16 out rounding; accumulation is fp32"
                            ):
                                nc.vector.reduce_sum(
                                    c_t[:, kc, n * nex : (n + 1) * nex],
                                    seg,
                                    axis=mybir.AxisListType.X,
                                )
                    # cwT[ex, feat] = c.T @ W_bot  (c is already [feat, ex] = lhsT)
                    pcw = pscw.tile([ST_EX, H], F32, tag="pcw")
                    for kc in range(2):
                        off = (k * 2 + kc) * H
                        nc.tensor.matmul(
                            pcw[:],
                            c_t[:, kc, :],
                            wbot_sb[:, off : off + H],
                            start=(kc == 0),
                            stop=(kc == 1),
                        )
                    cwT_sb = cpool.tile([ST_EX, H], F16, tag="cwT")
                    nc.vector.tensor_copy(cwT_sb[:], pcw[:])

                    # h' = tanh(W_top.T @ h + cw(bcast over agents via selector
                    # matmul) + b_h)
                    for m in range(2):
                        def comm_mm(out_ap, n, m=m, k=k):
                            for kc in range(2):
                                off = (k * 2 + kc) * H + m * 128
                                nc.tensor.matmul(
                                    out_ap,
                                    wtop_sb[:, off : off + 128],
                                    hcur[kc][:, n * SUB : (n + 1) * SUB],
                                    start=(kc == 0),
                                    stop=False,
                                )
                            nc.tensor.matmul(
                                out_ap,
                                cwT_sb[:, m * 128 : (m + 1) * 128],
                                sel_sb[:, n * SUB : (n + 1) * SUB],
                                start=False,
                                stop=True,
                            )
                        for grp in TANH_GROUPS:
                            tanh_grouped(
                                m, grp, comm_mm, hnxt[m],
                                bh_sb[:, k * 2 + m : k * 2 + m + 1],
                            )
                    hcur, hnxt = hnxt, hcur

                # decoder: y = W_dec.T @ h + b_dec
                out_t = opool.tile([DOUT, ST], F32, tag="out")
                for n in range(NSUB):
                    pd = psdec.tile([DOUT, SUB], F32, tag="pd")
                    for kc in range(2):
                        nc.tensor.matmul(
                            pd[:],
                            wdec_sb[:, kc * DOUT : (kc + 1) * DOUT],
                            hcur[kc][:, n * SUB : (n + 1) * SUB],
                            start=(kc == 0),
                            stop=(kc == 1),
                        )
                    nc.vector.tensor_scalar_add(
                        out_t[:, n * SUB : (n + 1) * SUB], pd[:], bdec_sb[:, 0:1]
                    )
                nc.sync.dma_start(y[:, t0 : t0 + ST], out_t[:])

    nc.compile()
    return nc


def host_inputs(x, W_enc, b_enc, W_h, b_h, W_dec, b_dec, n_cores=N_CORES, bs=BS):
    """Shard x over cores (pre-transposed to [DIN, tok]); replicate weights."""
    x = np.asarray(x, np.float32)
    common = {
        "w_enc": np.ascontiguousarray(np.asarray(W_enc, np.float32)),
        "b_enc": np.ascontiguousarray(
            np.asarray(b_enc, np.float32).reshape(2, 128).T
        ),
        "w_top": np.ascontiguousarray(
            np.asarray(W_h, np.float32)[:, :H, :].reshape(K, 2, 128, H)
        ),
        "w_bot": np.ascontiguousarray(
            (np.asarray(W_h, np.float32)[:, H:, :] / A).reshape(K, 2, 128, H)
        ),
        "b_h": np.ascontiguousarray(
            np.asarray(b_h, np.float32).reshape(K, 2, 128).transpose(2, 0, 1).reshape(128, K * 2)
        ),
        "w_dec": np.ascontiguousarray(
            np.asarray(W_dec, np.float32).reshape(2, 128, DOUT)
        ),
        "b_dec": np.ascontiguousarray(np.asarray(b_dec, np.float32).reshape(DOUT, 1)),
        "sel": np.ascontiguousarray(
            np.repeat(np.eye(ST_EX, dtype=np.float32), A, axis=1)
        ),
    }
    in_maps = []
    for i in range(n_cores):
        shard = x[i * bs : (i + 1) * bs].reshape(bs * A, DIN)
        in_maps.append({**common, "xT": np.ascontiguousarray(shard.T)})
    return in_maps


_NC_CACHE = None


def _get_nc():
    global _NC_CACHE
    if _NC_CACHE is None:
        _NC_CACHE = build_nc()
    return _NC_CACHE


def kernel(x, W_enc, b_enc, W_h, b_h, W_dec, b_dec, _run_kwargs=None):
    in_maps = host_inputs(x, W_enc, b_enc, W_h, b_h, W_dec, b_dec)
    nc = _get_nc()
    res = run_bass_kernel_spmd(nc, in_maps, list(range(N_CORES)), **(_run_kwargs or {}))
    outs = [
        res.results[i]["y"].T.reshape(BS, A, DOUT).astype(np.float32)
        for i in range(N_CORES)
    ]
    full = np.concatenate(outs, axis=0)
    if _run_kwargs:
        kernel.last_results = res
    return full


# revision 11
# speedup vs baseline: 1.4655x; 1.4655x over previous
"""CommNet (B=4096, A=50, DIN=128, H=256, DOUT=64, K=2) on 8 TRN2 NeuronCores.

Data-parallel over the batch axis: 512 examples (25600 agent-tokens) per core,
weights replicated. On-chip layout is feature-major ([feature, token]) so every
layer's contraction dim sits on SBUF partitions; the host pre-transposes each
x shard once (numpy) so no on-chip transposes are needed.

Per comm step the concat [h, c] @ W is split as h @ W_top + c @ W_bot with the
1/50 agent-mean folded into W_bot on the host. The per-example c @ W_bot result
(computed transposed, with c as the stationary matmul operand) is broadcast
back over agents by a third accumulating matmul against a constant 0/1
selector, so the whole comm step stays on the PE and lands in one PSUM tile.

Matmul operands are fp16 (1 cyc/row, fast weight load; PSUM accumulation is
fp32). tanh runs on ScalarE over three bank-aligned PSUM sub-tiles per
ACTIVATE (N=1200) to amortize the per-op ~352-cycle overhead. VectorE does the
per-example agent-sum reductions (fp16 2x mode) and the decoder bias-add;
GpSimd only drives the casting DMAs.
"""

import numpy as np

import concourse.bacc as bacc
import concourse.bass as bass
import concourse.tile as tile
from concourse import mybir
from concourse.bass_utils import run_bass_kernel_spmd

N_CORES = 8
B, A, DIN, H, DOUT, K = 4096, 50, 128, 256, 64, 2
BS = B // N_CORES          # examples per core
TOK = BS * A               # tokens per core
ST_EX = 64                 # examples per supertile
ST = ST_EX * A             # 3200 tokens per supertile
SUB_EX = 8                 # examples per matmul sub-tile
SUB = SUB_EX * A           # 400 tokens (PSUM bank limit: N <= 512 fp32 accum)
NSUB = ST // SUB           # 8
BANK = 512                 # fp32 elems per PSUM bank
RSUB = 1600                # tokens per DVE reduce op (32 examples)

F32 = mybir.dt.float32
F16 = mybir.dt.float16
Tanh = mybir.ActivationFunctionType.Tanh

# tanh batching: groups of matmul sub-tiles sharing one PSUM tile + ACTIVATE
TANH_GROUPS = [(0, 1, 2), (3, 4, 5), (6, 7)]


def build_nc(n_supertiles=BS // ST_EX):
    tok = n_supertiles * ST
    nc = bacc.Bacc(
        "TRN2",
        target_bir_lowering=False,
        debug=False,
        enable_asserts=True,
        num_devices=N_CORES,
    )
    xT = nc.dram_tensor("xT", [DIN, tok], F32, kind="ExternalInput")
    w_enc = nc.dram_tensor("w_enc", [DIN, H], F32, kind="ExternalInput")
    b_enc = nc.dram_tensor("b_enc", [128, 2], F32, kind="ExternalInput")
    w_top = nc.dram_tensor("w_top", [K, 2, 128, H], F32, kind="ExternalInput")
    w_bot = nc.dram_tensor("w_bot", [K, 2, 128, H], F32, kind="ExternalInput")
    b_h = nc.dram_tensor("b_h", [128, K * 2], F32, kind="ExternalInput")
    w_dec = nc.dram_tensor("w_dec", [2, 128, DOUT], F32, kind="ExternalInput")
    b_dec = nc.dram_tensor("b_dec", [DOUT, 1], F32, kind="ExternalInput")
    sel = nc.dram_tensor("sel", [ST_EX, ST], F32, kind="ExternalInput")
    y = nc.dram_tensor("y", [DOUT, tok], F32, kind="ExternalOutput")

    with tile.TileContext(nc) as tc:
        with (
            tc.tile_pool(name="wpool", bufs=1) as wpool,
            tc.tile_pool(name="xpool", bufs=2) as xpool,
            tc.tile_pool(name="hpool", bufs=2) as hpool,
            tc.tile_pool(name="opool", bufs=2) as opool,
            tc.tile_pool(name="cpool", bufs=2) as cpool,
            tc.tile_pool(name="psmm", bufs=2, space=bass.MemorySpace.PSUM) as psmm,
            tc.tile_pool(name="psdec", bufs=1, space=bass.MemorySpace.PSUM) as psdec,
            tc.tile_pool(name="pscw", bufs=1, space=bass.MemorySpace.PSUM) as pscw,
        ):
            # --- weights: casting DMAs (f32 -> fp16), resident for the run ---
            wenc_sb = wpool.tile([DIN, H], F16)
            nc.gpsimd.dma_start(wenc_sb[:], w_enc[:])
            benc_sb = wpool.tile([128, 2], F32)
            nc.sync.dma_start(benc_sb[:], b_enc[:])
            wtop_sb = wpool.tile([128, K * 2 * H], F16)
            wbot_sb = wpool.tile([128, K * 2 * H], F16)
            for k in range(K):
                for kc in range(2):
                    off = (k * 2 + kc) * H
                    nc.gpsimd.dma_start(wtop_sb[:, off : off + H], w_top[k, kc])
                    nc.gpsimd.dma_start(wbot_sb[:, off : off + H], w_bot[k, kc])
            bh_sb = wpool.tile([128, K * 2], F32)
            nc.sync.dma_start(bh_sb[:], b_h[:])
            wdec_sb = wpool.tile([128, 2 * DOUT], F16)
            for kc in range(2):
                nc.gpsimd.dma_start(wdec_sb[:, kc * DOUT : (kc + 1) * DOUT], w_dec[kc])
            bdec_sb = wpool.tile([DOUT, 1], F32)
            nc.sync.dma_start(bdec_sb[:], b_dec[:])
            sel_sb = wpool.tile([ST_EX, ST], F16)
            nc.gpsimd.dma_start(sel_sb[:], sel[:])

            for s in range(n_supertiles):
                t0 = s * ST
                xt = xpool.tile([DIN, ST], F16, tag="xt")
                nc.gpsimd.dma_start(xt[:], xT[:, t0 : t0 + ST])
                hA = [hpool.tile([128, ST], F16, tag=f"hA{m}", name=f"hA{m}_{s}") for m in range(2)]
                hB = [hpool.tile([128, ST], F16, tag=f"hB{m}", name=f"hB{m}_{s}") for m in range(2)]

                def psum_view(ps, g):
                    # [128, g, SUB] strided view of the first g banks
                    return ps[:].rearrange("p (g b) -> p g b", b=BANK)[:, 0:g, 0:SUB]

                def tanh_grouped(m, grp, mm_emit, hout, bias, c_out=None):
                    # per-sub-tile matmul groups land bank-aligned in one
                    # multi-bank PSUM tile; one strided tanh covers them all;
                    # optionally an incremental agent-sum of the fresh tanh
                    # output follows so the cwT chain never stalls the PE
                    g = len(grp)
                    ps = psmm.tile(
                        [128, 3 * BANK], F32, tag="ps", name=f"ps_{s}_{m}_{grp[0]}"
                    )
                    for j, n in enumerate(grp):
                        mm_emit(ps[:, j * BANK : j * BANK + SUB], n)
                    lo, hi = grp[0] * SUB, (grp[-1] + 1) * SUB
                    hv = hout[:, lo:hi].rearrange("p (g b) -> p g b", b=SUB)
                    nc.scalar.activation(hv, psum_view(ps, g), Tanh, bias=bias)
                    if c_out is not None:
                        seg = hout[:, lo:hi].rearrange("p (b a) -> p b a", a=A)
                        with nc.allow_low_precision(
                            reason="fp16 out rounding; accumulation is fp32"
                        ):
                            nc.vector.reduce_sum(
                                c_out[
                                    :, m, grp[0] * SUB_EX : (grp[-1] + 1) * SUB_EX
                                ],
                                seg,
                                axis=mybir.AxisListType.X,
                            )

                # encoder: h = tanh(W_enc.T @ xT + b_enc); agent-sums of hA
                # ride along incrementally into c_ts[0]
                c_ts = [
                    cpool.tile([128, 2, ST_EX], F16, tag=f"c{k}", name=f"c{k}_{s}")
                    for k in range(K)
                ]
                for m in range(2):
                    def enc_mm(out_ap, n, m=m):
                        nc.tensor.matmul(
                            out_ap,
                            wenc_sb[:, m * 128 : (m + 1) * 128],
                            xt[:, n * SUB : (n + 1) * SUB],
                            start=True,
                            stop=True,
                        )
                    for grp in TANH_GROUPS:
                        tanh_grouped(
                            m, grp, enc_mm, hA[m], benc_sb[:, m : m + 1],
                            c_out=c_ts[0],
                        )

                hcur, hnxt = hA, hB
                for k in range(K):
                    c_t = c_ts[k]
                    # cwT[ex, feat] = c.T @ W_bot  (c is already [feat, ex] = lhsT)
                    pcw = pscw.tile([ST_EX, H], F32, tag="pcw")
                    for kc in range(2):
                        off = (k * 2 + kc) * H
                        nc.tensor.matmul(
                            pcw[:],
                            c_t[:, kc, :],
                            wbot_sb[:, off : off + H],
                            start=(kc == 0),
                            stop=(kc == 1),
                        )
                    cwT_sb = cpool.tile([ST_EX, H], F16, tag="cwT")
                    nc.vector.tensor_copy(cwT_sb[:], pcw[:])

                    # h' = tanh(W_top.T @ h + cw(bcast over agents via selector
                    # matmul) + b_h)
                    for m in range(2):
                        for grp in TANH_GROUPS:
                            g = len(grp)
                            ps = psmm.tile(
                                [128, 3 * BANK], F32, tag="ps",
                                name=f"psc_{s}_{k}_{m}_{grp[0]}",
                            )
                            # same stationary weight across each j-sweep so
                            # consecutive LDWEIGHTS are redundant/overlappable
                            for kc in range(2):
                                off = (k * 2 + kc) * H + m * 128
                                for j, n in enumerate(grp):
                                    nc.tensor.matmul(
                                        ps[:, j * BANK : j * BANK + SUB],
                                        wtop_sb[:, off : off + 128],
                                        hcur[kc][:, n * SUB : (n + 1) * SUB],
                                        start=(kc == 0),
                                        stop=False,
                                    )
                            for j, n in enumerate(grp):
                                nc.tensor.matmul(
                                    ps[:, j * BANK : j * BANK + SUB],
                                    cwT_sb[:, m * 128 : (m + 1) * 128],
                                    sel_sb[:, n * SUB : (n + 1) * SUB],
                                    start=False,
                                    stop=True,
                                )
                            lo, hi = grp[0] * SUB, (grp[-1] + 1) * SUB
                            hv = hnxt[m][:, lo:hi].rearrange(
                                "p (g b) -> p g b", b=SUB
                            )
                            nc.scalar.activation(
                                hv, psum_view(ps, g), Tanh,
                                bias=bh_sb[:, k * 2 + m : k * 2 + m + 1],
                            )
                            if k + 1 < K:
                                seg = hnxt[m][:, lo:hi].rearrange(
                                    "p (b a) -> p b a", a=A
                                )
                                with nc.allow_low_precision(
                                    reason="fp16 out rounding; accum is fp32"
                                ):
                                    nc.vector.reduce_sum(
                                        c_ts[k + 1][
                                            :, m,
                                            grp[0] * SUB_EX
                                            : (grp[-1] + 1) * SUB_EX,
                                        ],
                                        seg,
                                        axis=mybir.AxisListType.X,
                                    )
                    hcur, hnxt = hnxt, hcur

                # decoder: y = W_dec.T @ h + b_dec
                out_t = opool.tile([DOUT, ST], F32, tag="out")
                for n in range(NSUB):
                    pd = psdec.tile([DOUT, SUB], F32, tag="pd")
                    for kc in range(2):
                        nc.tensor.matmul(
                            pd[:],
                            wdec_sb[:, kc * DOUT : (kc + 1) * DOUT],
                            hcur[kc][:, n * SUB : (n + 1) * SUB],
                            start=(kc == 0),
                            stop=(kc == 1),
                        )
                    nc.vector.tensor_scalar_add(
                        out_t[:, n * SUB : (n + 1) * SUB], pd[:], bdec_sb[:, 0:1]
                    )
                nc.sync.dma_start(y[:, t0 : t0 + ST], out_t[:])

    nc.compile()
    return nc


def host_inputs(x, W_enc, b_enc, W_h, b_h, W_dec, b_dec, n_cores=N_CORES, bs=BS):
    """Shard x over cores (pre-transposed to [DIN, tok]); replicate weights."""
    x = np.asarray(x, np.float32)
    common = {
        "w_enc": np.ascontiguousarray(np.asarray(W_enc, np.float32)),
        "b_enc": np.ascontiguousarray(
            np.asarray(b_enc, np.float32).reshape(2, 128).T
        ),
        "w_top": np.ascontiguousarray(
            np.asarray(W_h, np.float32)[:, :H, :].reshape(K, 2, 128, H)
        ),
        "w_bot": np.ascontiguousarray(
            (np.asarray(W_h, np.float32)[:, H:, :] / A).reshape(K, 2, 128, H)
        ),
        "b_h": np.ascontiguousarray(
            np.asarray(b_h, np.float32).reshape(K, 2, 128).transpose(2, 0, 1).reshape(128, K * 2)
        ),
        "w_dec": np.ascontiguousarray(
            np.asarray(W_dec, np.float32).reshape(2, 128, DOUT)
        ),
        "b_dec": np.ascontiguousarray(np.asarray(b_dec, np.float32).reshape(DOUT, 1)),
        "sel": np.ascontiguousarray(
            np.repeat(np.eye(ST_EX, dtype=np.float32), A, axis=1)
        ),
    }
    in_maps = []
    for i in range(n_cores):
        shard = x[i * bs : (i + 1) * bs].reshape(bs * A, DIN)
        in_maps.append({**common, "xT": np.ascontiguousarray(shard.T)})
    return in_maps


_NC_CACHE = None


def _get_nc():
    global _NC_CACHE
    if _NC_CACHE is None:
        _NC_CACHE = build_nc()
    return _NC_CACHE


def kernel(x, W_enc, b_enc, W_h, b_h, W_dec, b_dec, _run_kwargs=None):
    in_maps = host_inputs(x, W_enc, b_enc, W_h, b_h, W_dec, b_dec)
    nc = _get_nc()
    res = run_bass_kernel_spmd(nc, in_maps, list(range(N_CORES)), **(_run_kwargs or {}))
    outs = [
        res.results[i]["y"].T.reshape(BS, A, DOUT).astype(np.float32)
        for i in range(N_CORES)
    ]
    full = np.concatenate(outs, axis=0)
    if _run_kwargs:
        kernel.last_results = res
    return full


# revision 12
# speedup vs baseline: 1.5314x; 1.0449x over previous
"""CommNet (B=4096, A=50, DIN=128, H=256, DOUT=64, K=2) on 8 TRN2 NeuronCores.

Data-parallel over the batch axis: 512 examples (25600 agent-tokens) per core,
weights replicated. On-chip layout is feature-major ([feature, token]) so every
layer's contraction dim sits on SBUF partitions; the host pre-transposes each
x shard once (numpy) so no on-chip transposes are needed.

Per comm step the concat [h, c] @ W is split as h @ W_top + c @ W_bot with the
1/50 agent-mean folded into W_bot on the host. The per-example c @ W_bot result
(computed transposed, with c as the stationary matmul operand) is broadcast
back over agents by a third accumulating matmul against a constant 0/1
selector, so the whole comm step stays on the PE and lands in one PSUM tile.

Matmul operands are fp16 (1 cyc/row, fast weight load; PSUM accumulation is
fp32). tanh runs on ScalarE over three bank-aligned PSUM sub-tiles per
ACTIVATE (N=1200) to amortize the per-op ~352-cycle overhead. VectorE does the
per-example agent-sum reductions (fp16 2x mode) and the decoder bias-add;
GpSimd only drives the casting DMAs.
"""

import numpy as np

import concourse.bacc as bacc
import concourse.bass as bass
import concourse.tile as tile
from concourse import mybir
from concourse.bass_utils import run_bass_kernel_spmd

N_CORES = 8
B, A, DIN, H, DOUT, K = 4096, 50, 128, 256, 64, 2
BS = B // N_CORES          # examples per core
TOK = BS * A               # tokens per core
ST_EX = 64                 # examples per supertile
ST = ST_EX * A             # 3200 tokens per supertile
SUB_EX = 8                 # examples per matmul sub-tile
SUB = SUB_EX * A           # 400 tokens (PSUM bank limit: N <= 512 fp32 accum)
NSUB = ST // SUB           # 8
BANK = 512                 # fp32 elems per PSUM bank
RSUB = 1600                # tokens per DVE reduce op (32 examples)

F32 = mybir.dt.float32
F16 = mybir.dt.float16
Tanh = mybir.ActivationFunctionType.Tanh

# tanh batching: groups of matmul sub-tiles sharing one PSUM tile + ACTIVATE
TANH_GROUPS = [(0, 1), (2, 3), (4, 5), (6, 7)]


def build_nc(n_supertiles=BS // ST_EX):
    tok = n_supertiles * ST
    nc = bacc.Bacc(
        "TRN2",
        target_bir_lowering=False,
        debug=False,
        enable_asserts=True,
        num_devices=N_CORES,
    )
    xT = nc.dram_tensor("xT", [DIN, tok], F32, kind="ExternalInput")
    w_enc = nc.dram_tensor("w_enc", [DIN, H], F32, kind="ExternalInput")
    b_enc = nc.dram_tensor("b_enc", [128, 2], F32, kind="ExternalInput")
    w_top = nc.dram_tensor("w_top", [K, 2, 128, H], F32, kind="ExternalInput")
    w_bot = nc.dram_tensor("w_bot", [K, 2, 128, H], F32, kind="ExternalInput")
    b_h = nc.dram_tensor("b_h", [128, K * 2], F32, kind="ExternalInput")
    w_dec = nc.dram_tensor("w_dec", [2, 128, DOUT], F32, kind="ExternalInput")
    b_dec = nc.dram_tensor("b_dec", [DOUT, 1], F32, kind="ExternalInput")
    sel = nc.dram_tensor("sel", [ST_EX, ST], F32, kind="ExternalInput")
    y = nc.dram_tensor("y", [DOUT, tok], F32, kind="ExternalOutput")

    with tile.TileContext(nc) as tc:
        with (
            tc.tile_pool(name="wpool", bufs=1) as wpool,
            tc.tile_pool(name="xpool", bufs=2) as xpool,
            tc.tile_pool(name="hpool", bufs=2) as hpool,
            tc.tile_pool(name="opool", bufs=2) as opool,
            tc.tile_pool(name="cpool", bufs=2) as cpool,
            tc.tile_pool(name="psmm", bufs=3, space=bass.MemorySpace.PSUM) as psmm,
            tc.tile_pool(name="psdec", bufs=2, space=bass.MemorySpace.PSUM) as psdec,
        ):
            # --- weights: casting DMAs (f32 -> fp16), resident for the run ---
            wenc_sb = wpool.tile([DIN, H], F16)
            nc.gpsimd.dma_start(wenc_sb[:], w_enc[:])
            benc_sb = wpool.tile([128, 2], F32)
            nc.sync.dma_start(benc_sb[:], b_enc[:])
            wtop_sb = wpool.tile([128, K * 2 * H], F16)
            wbot_sb = wpool.tile([128, K * 2 * H], F16)
            for k in range(K):
                for kc in range(2):
                    off = (k * 2 + kc) * H
                    nc.gpsimd.dma_start(wtop_sb[:, off : off + H], w_top[k, kc])
                    nc.gpsimd.dma_start(wbot_sb[:, off : off + H], w_bot[k, kc])
            bh_sb = wpool.tile([128, K * 2], F32)
            nc.sync.dma_start(bh_sb[:], b_h[:])
            wdec_sb = wpool.tile([128, 2 * DOUT], F16)
            for kc in range(2):
                nc.gpsimd.dma_start(wdec_sb[:, kc * DOUT : (kc + 1) * DOUT], w_dec[kc])
            bdec_sb = wpool.tile([DOUT, 1], F32)
            nc.sync.dma_start(bdec_sb[:], b_dec[:])
            sel_sb = wpool.tile([ST_EX, ST], F16)
            nc.gpsimd.dma_start(sel_sb[:], sel[:])

            for s in range(n_supertiles):
                t0 = s * ST
                xt = xpool.tile([DIN, ST], F16, tag="xt")
                nc.gpsimd.dma_start(xt[:], xT[:, t0 : t0 + ST])
                hA = [hpool.tile([128, ST], F16, tag=f"hA{m}", name=f"hA{m}_{s}") for m in range(2)]
                hB = [hpool.tile([128, ST], F16, tag=f"hB{m}", name=f"hB{m}_{s}") for m in range(2)]

                def psum_view(ps, g):
                    # [128, g, SUB] strided view of the first g banks
                    return ps[:].rearrange("p (g b) -> p g b", b=BANK)[:, 0:g, 0:SUB]

                def tanh_grouped(m, grp, mm_emit, hout, bias, c_out=None):
                    # per-sub-tile matmul groups land bank-aligned in one
                    # multi-bank PSUM tile; one strided tanh covers them all;
                    # optionally an incremental agent-sum of the fresh tanh
                    # output follows so the cwT chain never stalls the PE
                    g = len(grp)
                    ps = psmm.tile(
                        [128, 2 * BANK], F32, tag="ps", name=f"ps_{s}_{m}_{grp[0]}"
                    )
                    for j, n in enumerate(grp):
                        mm_emit(ps[:, j * BANK : j * BANK + SUB], n)
                    lo, hi = grp[0] * SUB, (grp[-1] + 1) * SUB
                    hv = hout[:, lo:hi].rearrange("p (g b) -> p g b", b=SUB)
                    nc.scalar.activation(hv, psum_view(ps, g), Tanh, bias=bias)
                    if c_out is not None:
                        seg = hout[:, lo:hi].rearrange("p (b a) -> p b a", a=A)
                        with nc.allow_low_precision(
                            reason="fp16 out rounding; accumulation is fp32"
                        ):
                            nc.vector.reduce_sum(
                                c_out[
                                    :, m, grp[0] * SUB_EX : (grp[-1] + 1) * SUB_EX
                                ],
                                seg,
                                axis=mybir.AxisListType.X,
                            )

                # encoder: h = tanh(W_enc.T @ xT + b_enc); agent-sums of hA
                # ride along incrementally into c_ts[0]
                c_ts = [
                    cpool.tile([128, 2, ST_EX], F16, tag=f"c{k}", name=f"c{k}_{s}")
                    for k in range(K)
                ]
                for m in range(2):
                    def enc_mm(out_ap, n, m=m):
                        nc.tensor.matmul(
                            out_ap,
                            wenc_sb[:, m * 128 : (m + 1) * 128],
                            xt[:, n * SUB : (n + 1) * SUB],
                            start=True,
                            stop=True,
                        )
                    for grp in TANH_GROUPS:
                        tanh_grouped(
                            m, grp, enc_mm, hA[m], benc_sb[:, m : m + 1],
                            c_out=c_ts[0],
                        )

                hcur, hnxt = hA, hB
                for k in range(K):
                    c_t = c_ts[k]
                    # cwT[ex, feat] = c.T @ W_bot  (c is already [feat, ex] = lhsT)
                    pcw = psdec.tile([ST_EX, H], F32, tag="pd", name=f"pcw_{s}_{k}")
                    for kc in range(2):
                        off = (k * 2 + kc) * H
                        nc.tensor.matmul(
                            pcw[:],
                            c_t[:, kc, :],
                            wbot_sb[:, off : off + H],
                            start=(kc == 0),
                            stop=(kc == 1),
                        )
                    cwT_sb = cpool.tile([ST_EX, H], F16, tag="cwT")
                    nc.vector.tensor_copy(cwT_sb[:], pcw[:])

                    # h' = tanh(W_top.T @ h + cw(bcast over agents via selector
                    # matmul) + b_h)
                    for m in range(2):
                        for grp in TANH_GROUPS:
                            g = len(grp)
                            ps = psmm.tile(
                                [128, 2 * BANK], F32, tag="ps",
                                name=f"psc_{s}_{k}_{m}_{grp[0]}",
                            )
                            # same stationary weight across each j-sweep so
                            # consecutive LDWEIGHTS are redundant/overlappable
                            for kc in range(2):
                                off = (k * 2 + kc) * H + m * 128
                                for j, n in enumerate(grp):
                                    nc.tensor.matmul(
                                        ps[:, j * BANK : j * BANK + SUB],
                                        wtop_sb[:, off : off + 128],
                                        hcur[kc][:, n * SUB : (n + 1) * SUB],
                                        start=(kc == 0),
                                        stop=False,
                                    )
                            for j, n in enumerate(grp):
                                nc.tensor.matmul(
                                    ps[:, j * BANK : j * BANK + SUB],
                                    cwT_sb[:, m * 128 : (m + 1) * 128],
                                    sel_sb[:, n * SUB : (n + 1) * SUB],
                                    start=False,
                                    stop=True,
                                )
                            lo, hi = grp[0] * SUB, (grp[-1] + 1) * SUB
                            hv = hnxt[m][:, lo:hi].rearrange(
                                "p (g b) -> p g b", b=SUB
                            )
                            nc.scalar.activation(
                                hv, psum_view(ps, g), Tanh,
                                bias=bh_sb[:, k * 2 + m : k * 2 + m + 1],
                            )
                            if k + 1 < K:
                                seg = hnxt[m][:, lo:hi].rearrange(
                                    "p (b a) -> p b a", a=A
                                )
                                with nc.allow_low_precision(
                                    reason="fp16 out rounding; accum is fp32"
                                ):
                                    nc.vector.reduce_sum(
                                        c_ts[k + 1][
                                            :, m,
                                            grp[0] * SUB_EX
                                            : (grp[-1] + 1) * SUB_EX,
                                        ],
                                        seg,
                                        axis=mybir.AxisListType.X,
                                    )
                    hcur, hnxt = hnxt, hcur

                # decoder: y = W_dec.T @ h + b_dec
                out_t = opool.tile([DOUT, ST], F32, tag="out")
                for n in range(NSUB):
                    pd = psdec.tile([DOUT, SUB], F32, tag="pd")
                    for kc in range(2):
                        nc.tensor.matmul(
                            pd[:],
                            wdec_sb[:, kc * DOUT : (kc + 1) * DOUT],
                            hcur[kc][:, n * SUB : (n + 1) * SUB],
                            start=(kc == 0),
                            stop=(kc == 1),
                        )
                    nc.vector.tensor_scalar_add(
                        out_t[:, n * SUB : (n + 1) * SUB], pd[:], bdec_sb[:, 0:1]
                    )
                nc.sync.dma_start(y[:, t0 : t0 + ST], out_t[:])

    nc.compile()
    return nc


def host_inputs(x, W_enc, b_enc, W_h, b_h, W_dec, b_dec, n_cores=N_CORES, bs=BS):
    """Shard x over cores (pre-transposed to [DIN, tok]); replicate weights."""
    x = np.asarray(x, np.float32)
    common = {
        "w_enc": np.ascontiguousarray(np.asarray(W_enc, np.float32)),
        "b_enc": np.ascontiguousarray(
            np.asarray(b_enc, np.float32).reshape(2, 128).T
        ),
        "w_top": np.ascontiguousarray(
            np.asarray(W_h, np.float32)[:, :H, :].reshape(K, 2, 128, H)
        ),
        "w_bot": np.ascontiguousarray(
            (np.asarray(W_h, np.float32)[:, H:, :] / A).reshape(K, 2, 128, H)
        ),
        "b_h": np.ascontiguousarray(
            np.asarray(b_h, np.float32).reshape(K, 2, 128).transpose(2, 0, 1).reshape(128, K * 2)
        ),
        "w_dec": np.ascontiguousarray(
            np.asarray(W_dec, np.float32).reshape(2, 128, DOUT)
        ),
        "b_dec": np.ascontiguousarray(np.asarray(b_dec, np.float32).reshape(DOUT, 1)),
        "sel": np.ascontiguousarray(
            np.repeat(np.eye(ST_EX, dtype=np.float32), A, axis=1)
        ),
    }
    in_maps = []
    for i in range(n_cores):
        shard = x[i * bs : (i + 1) * bs].reshape(bs * A, DIN)
        in_maps.append({**common, "xT": np.ascontiguousarray(shard.T)})
    return in_maps


_NC_CACHE = None


def _get_nc():
    global _NC_CACHE
    if _NC_CACHE is None:
        _NC_CACHE = build_nc()
    return _NC_CACHE


def kernel(x, W_enc, b_enc, W_h, b_h, W_dec, b_dec, _run_kwargs=None):
    in_maps = host_inputs(x, W_enc, b_enc, W_h, b_h, W_dec, b_dec)
    nc = _get_nc()
    res = run_bass_kernel_spmd(nc, in_maps, list(range(N_CORES)), **(_run_kwargs or {}))
    outs = [
        res.results[i]["y"].T.reshape(BS, A, DOUT).astype(np.float32)
        for i in range(N_CORES)
    ]
    full = np.concatenate(outs, axis=0)
    if _run_kwargs:
        kernel.last_results = res
    return full


# revision 13
# speedup vs baseline: 1.6394x; 1.0705x over previous
"""CommNet (B=4096, A=50, DIN=128, H=256, DOUT=64, K=2) on 8 TRN2 NeuronCores.

Data-parallel over the batch axis: 512 examples (25600 agent-tokens) per core,
weights replicated. On-chip layout is feature-major ([feature, token]) so every
layer's contraction dim sits on SBUF partitions; the host pre-transposes each
x shard once (numpy) so no on-chip transposes are needed.

Per comm step the concat [h, c] @ W is split as h @ W_top + c @ W_bot with the
1/50 agent-mean folded into W_bot on the host. The per-example c @ W_bot result
(computed transposed, with c as the stationary matmul operand) is broadcast
back over agents by a third accumulating matmul against a constant 0/1
selector, so the whole comm step stays on the PE and lands in one PSUM tile.

Matmul operands are fp16 (1 cyc/row, fast weight load; PSUM accumulation is
fp32). tanh runs on ScalarE over three bank-aligned PSUM sub-tiles per
ACTIVATE (N=1200) to amortize the per-op ~352-cycle overhead. VectorE does the
per-example agent-sum reductions (fp16 2x mode) and the decoder bias-add;
GpSimd only drives the casting DMAs.
"""

import numpy as np

import concourse.bacc as bacc
import concourse.bass as bass
import concourse.tile as tile
from concourse import mybir
from concourse.bass_utils import run_bass_kernel_spmd

N_CORES = 8
B, A, DIN, H, DOUT, K = 4096, 50, 128, 256, 64, 2
BS = B // N_CORES          # examples per core
TOK = BS * A               # tokens per core
ST_EX = 64                 # examples per supertile
ST = ST_EX * A             # 3200 tokens per supertile
SUB_EX = 8                 # examples per matmul sub-tile
SUB = SUB_EX * A           # 400 tokens (PSUM bank limit: N <= 512 fp32 accum)
NSUB = ST // SUB           # 8
BANK = 512                 # fp32 elems per PSUM bank
RSUB = 1600                # tokens per DVE reduce op (32 examples)

F32 = mybir.dt.float32
F16 = mybir.dt.float16
Tanh = mybir.ActivationFunctionType.Tanh

# tanh batching: groups of matmul sub-tiles sharing one PSUM tile + ACTIVATE
TANH_GROUPS = [(0, 1), (2, 3), (4, 5), (6, 7)]


def build_nc(n_supertiles=BS // ST_EX):
    tok = n_supertiles * ST
    nc = bacc.Bacc(
        "TRN2",
        target_bir_lowering=False,
        debug=False,
        enable_asserts=True,
        num_devices=N_CORES,
    )
    xT = nc.dram_tensor("xT", [DIN, tok], F32, kind="ExternalInput")
    w_enc = nc.dram_tensor("w_enc", [DIN, H], F32, kind="ExternalInput")
    b_enc = nc.dram_tensor("b_enc", [128, 2], F32, kind="ExternalInput")
    w_top = nc.dram_tensor("w_top", [K, 2, 128, H], F32, kind="ExternalInput")
    w_bot = nc.dram_tensor("w_bot", [K, 2, 128, H], F32, kind="ExternalInput")
    b_h = nc.dram_tensor("b_h", [128, K * 2], F32, kind="ExternalInput")
    w_dec = nc.dram_tensor("w_dec", [2, 128, DOUT], F32, kind="ExternalInput")
    b_dec = nc.dram_tensor("b_dec", [DOUT, 1], F32, kind="ExternalInput")
    sel = nc.dram_tensor("sel", [ST_EX, ST], F32, kind="ExternalInput")
    y = nc.dram_tensor("y", [DOUT, tok], F32, kind="ExternalOutput")

    with tile.TileContext(nc) as tc:
        with (
            tc.tile_pool(name="wpool", bufs=1) as wpool,
            tc.tile_pool(name="xpool", bufs=4) as xpool,
            tc.tile_pool(name="hpool", bufs=4) as hpool,
            tc.tile_pool(name="opool", bufs=2) as opool,
            tc.tile_pool(name="cpool", bufs=4) as cpool,
            tc.tile_pool(name="psmm", bufs=3, space=bass.MemorySpace.PSUM) as psmm,
            tc.tile_pool(name="psdec", bufs=2, space=bass.MemorySpace.PSUM) as psdec,
        ):
            # --- weights: casting DMAs (f32 -> fp16), resident for the run ---
            wenc_sb = wpool.tile([DIN, H], F16)
            nc.gpsimd.dma_start(wenc_sb[:], w_enc[:])
            benc_sb = wpool.tile([128, 2], F32)
            nc.sync.dma_start(benc_sb[:], b_enc[:])
            wtop_sb = wpool.tile([128, K * 2 * H], F16)
            wbot_sb = wpool.tile([128, K * 2 * H], F16)
            for k in range(K):
                for kc in range(2):
                    off = (k * 2 + kc) * H
                    nc.gpsimd.dma_start(wtop_sb[:, off : off + H], w_top[k, kc])
                    nc.gpsimd.dma_start(wbot_sb[:, off : off + H], w_bot[k, kc])
            bh_sb = wpool.tile([128, K * 2], F32)
            nc.sync.dma_start(bh_sb[:], b_h[:])
            wdec_sb = wpool.tile([128, 2 * DOUT], F16)
            for kc in range(2):
                nc.gpsimd.dma_start(wdec_sb[:, kc * DOUT : (kc + 1) * DOUT], w_dec[kc])
            bdec_sb = wpool.tile([DOUT, 1], F32)
            nc.sync.dma_start(bdec_sb[:], b_dec[:])
            sel_sb = wpool.tile([ST_EX, ST], F16)
            nc.gpsimd.dma_start(sel_sb[:], sel[:])

            ILV = 4  # supertiles emitted in interleaved phase groups

            def psum_view(ps, g):
                return ps[:].rearrange("p (g b) -> p g b", b=BANK)[:, 0:g, 0:SUB]

            def reduce_seg(hout, lo, hi, c_out, m, g0, g1):
                seg = hout[:, lo:hi].rearrange("p (b a) -> p b a", a=A)
                with nc.allow_low_precision(
                    reason="fp16 out rounding; accumulation is fp32"
                ):
                    nc.vector.reduce_sum(
                        c_out[:, m, g0 * SUB_EX : g1 * SUB_EX],
                        seg,
                        axis=mybir.AxisListType.X,
                    )

            def make_state(s):
                xt = xpool.tile([DIN, ST], F16, tag="xt", name=f"xt_{s}")
                nc.gpsimd.dma_start(xt[:], xT[:, s * ST : (s + 1) * ST])
                hA = [
                    hpool.tile([128, ST], F16, tag=f"hA{m}", name=f"hA{m}_{s}")
                    for m in range(2)
                ]
                hB = [
                    hpool.tile([128, ST], F16, tag=f"hB{m}", name=f"hB{m}_{s}")
                    for m in range(2)
                ]
                c_ts = [
                    cpool.tile([128, 2, ST_EX], F16, tag=f"c{k}", name=f"c{k}_{s}")
                    for k in range(K)
                ]
                return {"s": s, "xt": xt, "hA": hA, "hB": hB, "c": c_ts}

            def enc_phase(st):
                s, xt, hA = st["s"], st["xt"], st["hA"]
                for m in range(2):
                    for grp in TANH_GROUPS:
                        g = len(grp)
                        ps = psmm.tile(
                            [128, 2 * BANK], F32, tag="ps", name=f"pse_{s}_{m}_{grp[0]}"
                        )
                        for j, n in enumerate(grp):
                            nc.tensor.matmul(
                                ps[:, j * BANK : j * BANK + SUB],
                                wenc_sb[:, m * 128 : (m + 1) * 128],
                                xt[:, n * SUB : (n + 1) * SUB],
                                start=True,
                                stop=True,
                            )
                        lo, hi = grp[0] * SUB, (grp[-1] + 1) * SUB
                        hv = hA[m][:, lo:hi].rearrange("p (g b) -> p g b", b=SUB)
                        nc.scalar.activation(
                            hv, psum_view(ps, g), Tanh, bias=benc_sb[:, m : m + 1]
                        )
                        reduce_seg(hA[m], lo, hi, st["c"][0], m, grp[0], grp[-1] + 1)

            def comm_phase(st, k):
                s, c_t = st["s"], st["c"][k]
                hcur = st["hA"] if k == 0 else st["hB"]
                hnxt = st["hB"] if k == 0 else st["hA"]
                # cwT[ex, feat] = c.T @ W_bot (c is already [feat, ex] = lhsT)
                pcw = psdec.tile([ST_EX, H], F32, tag="pd", name=f"pcw_{s}_{k}")
                for kc in range(2):
                    off = (k * 2 + kc) * H
                    nc.tensor.matmul(
                        pcw[:],
                        c_t[:, kc, :],
                        wbot_sb[:, off : off + H],
                        start=(kc == 0),
                        stop=(kc == 1),
                    )
                cwT_sb = cpool.tile([ST_EX, H], F16, tag="cwT", name=f"cwT_{s}_{k}")
                nc.vector.tensor_copy(cwT_sb[:], pcw[:])
                # h' = tanh(W_top.T @ h + cw(bcast via selector matmul) + b_h)
                for m in range(2):
                    for grp in TANH_GROUPS:
                        g = len(grp)
                        ps = psmm.tile(
                            [128, 2 * BANK], F32, tag="ps",
                            name=f"psc_{s}_{k}_{m}_{grp[0]}",
                        )
                        for kc in range(2):
                            off = (k * 2 + kc) * H + m * 128
                            for j, n in enumerate(grp):
                                nc.tensor.matmul(
                                    ps[:, j * BANK : j * BANK + SUB],
                                    wtop_sb[:, off : off + 128],
                                    hcur[kc][:, n * SUB : (n + 1) * SUB],
                                    start=(kc == 0),
                                    stop=False,
                                )
                        for j, n in enumerate(grp):
                            nc.tensor.matmul(
                                ps[:, j * BANK : j * BANK + SUB],
                                cwT_sb[:, m * 128 : (m + 1) * 128],
                                sel_sb[:, n * SUB : (n + 1) * SUB],
                                start=False,
                                stop=True,
                            )
                        lo, hi = grp[0] * SUB, (grp[-1] + 1) * SUB
                        hv = hnxt[m][:, lo:hi].rearrange("p (g b) -> p g b", b=SUB)
                        nc.scalar.activation(
                            hv, psum_view(ps, g), Tanh,
                            bias=bh_sb[:, k * 2 + m : k * 2 + m + 1],
                        )
                        if k + 1 < K:
                            reduce_seg(
                                hnxt[m], lo, hi, st["c"][k + 1], m,
                                grp[0], grp[-1] + 1,
                            )

            def dec_phase(st):
                s = st["s"]
                hcur = st["hA"] if K % 2 == 0 else st["hB"]
                out_t = opool.tile([DOUT, ST], F32, tag="out", name=f"out_{s}")
                for n in range(NSUB):
                    pd = psdec.tile([DOUT, SUB], F32, tag="pd", name=f"pd_{s}_{n}")
                    for kc in range(2):
                        nc.tensor.matmul(
                            pd[:],
                            wdec_sb[:, kc * DOUT : (kc + 1) * DOUT],
                            hcur[kc][:, n * SUB : (n + 1) * SUB],
                            start=(kc == 0),
                            stop=(kc == 1),
                        )
                    nc.vector.tensor_scalar_add(
                        out_t[:, n * SUB : (n + 1) * SUB], pd[:], bdec_sb[:, 0:1]
                    )
                nc.sync.dma_start(y[:, s * ST : (s + 1) * ST], out_t[:])

            assert n_supertiles % ILV == 0 or n_supertiles < ILV
            step = min(ILV, n_supertiles)
            for s0 in range(0, n_supertiles, step):
                sts = [make_state(s0 + i) for i in range(step)]
                for st in sts:
                    enc_phase(st)
                for k in range(K):
                    for st in sts:
                        comm_phase(st, k)
                for st in sts:
                    dec_phase(st)

    nc.compile()
    return nc


def host_inputs(x, W_enc, b_enc, W_h, b_h, W_dec, b_dec, n_cores=N_CORES, bs=BS):
    """Shard x over cores (pre-transposed to [DIN, tok]); replicate weights."""
    x = np.asarray(x, np.float32)
    common = {
        "w_enc": np.ascontiguousarray(np.asarray(W_enc, np.float32)),
        "b_enc": np.ascontiguousarray(
            np.asarray(b_enc, np.float32).reshape(2, 128).T
        ),
        "w_top": np.ascontiguousarray(
            np.asarray(W_h, np.float32)[:, :H, :].reshape(K, 2, 128, H)
        ),
        "w_bot": np.ascontiguousarray(
            (np.asarray(W_h, np.float32)[:, H:, :] / A).reshape(K, 2, 128, H)
        ),
        "b_h": np.ascontiguousarray(
            np.asarray(b_h, np.float32).reshape(K, 2, 128).transpose(2, 0, 1).reshape(128, K * 2)
        ),
        "w_dec": np.ascontiguousarray(
            np.asarray(W_dec, np.float32).reshape(2, 128, DOUT)
        ),
        "b_dec": np.ascontiguousarray(np.asarray(b_dec, np.float32).reshape(DOUT, 1)),
        "sel": np.ascontiguousarray(
            np.repeat(np.eye(ST_EX, dtype=np.float32), A, axis=1)
        ),
    }
    in_maps = []
    for i in range(n_cores):
        shard = x[i * bs : (i + 1) * bs].reshape(bs * A, DIN)
        in_maps.append({**common, "xT": np.ascontiguousarray(shard.T)})
    return in_maps


_NC_CACHE = None


def _get_nc():
    global _NC_CACHE
    if _NC_CACHE is None:
        _NC_CACHE = build_nc()
    return _NC_CACHE


def kernel(x, W_enc, b_enc, W_h, b_h, W_dec, b_dec, _run_kwargs=None):
    in_maps = host_inputs(x, W_enc, b_enc, W_h, b_h, W_dec, b_dec)
    nc = _get_nc()
    res = run_bass_kernel_spmd(nc, in_maps, list(range(N_CORES)), **(_run_kwargs or {}))
    outs = [
        res.results[i]["y"].T.reshape(BS, A, DOUT).astype(np.float32)
        for i in range(N_CORES)
    ]
    full = np.concatenate(outs, axis=0)
    if _run_kwargs:
        kernel.last_results = res
    return full


# revision 14
# speedup vs baseline: 1.6549x; 1.0095x over previous
"""CommNet (B=4096, A=50, DIN=128, H=256, DOUT=64, K=2) on 8 TRN2 NeuronCores.

Data-parallel over the batch axis: 512 examples (25600 agent-tokens) per core,
weights replicated. On-chip layout is feature-major ([feature, token]) so every
layer's contraction dim sits on SBUF partitions; the host pre-transposes each
x shard once (numpy) so no on-chip transposes are needed.

Per comm step the concat [h, c] @ W is split as h @ W_top + c @ W_bot with the
1/50 agent-mean folded into W_bot on the host. The per-example c @ W_bot result
(computed transposed, with c as the stationary matmul operand) is broadcast
back over agents by a third accumulating matmul against a constant 0/1
selector, so the whole comm step stays on the PE and lands in one PSUM tile.

Matmul operands are fp16 (1 cyc/row, fast weight load; PSUM accumulation is
fp32). tanh runs on ScalarE over three bank-aligned PSUM sub-tiles per
ACTIVATE (N=1200) to amortize the per-op ~352-cycle overhead. VectorE does the
per-example agent-sum reductions (fp16 2x mode) and the decoder bias-add;
GpSimd only drives the casting DMAs.
"""

import numpy as np

import concourse.bacc as bacc
import concourse.bass as bass
import concourse.tile as tile
from concourse import mybir
from concourse.bass_utils import run_bass_kernel_spmd

N_CORES = 8
B, A, DIN, H, DOUT, K = 4096, 50, 128, 256, 64, 2
BS = B // N_CORES          # examples per core
TOK = BS * A               # tokens per core
ST_EX = 64                 # examples per supertile
ST = ST_EX * A             # 3200 tokens per supertile
SUB_EX = 8                 # examples per matmul sub-tile
SUB = SUB_EX * A           # 400 tokens (PSUM bank limit: N <= 512 fp32 accum)
NSUB = ST // SUB           # 8
BANK = 512                 # fp32 elems per PSUM bank
RSUB = 1600                # tokens per DVE reduce op (32 examples)

F32 = mybir.dt.float32
F16 = mybir.dt.float16
Tanh = mybir.ActivationFunctionType.Tanh

# tanh batching: groups of matmul sub-tiles sharing one PSUM tile + ACTIVATE
TANH_GROUPS = [(0, 1), (2, 3), (4, 5), (6, 7)]


def build_nc(n_supertiles=BS // ST_EX):
    tok = n_supertiles * ST
    nc = bacc.Bacc(
        "TRN2",
        target_bir_lowering=False,
        debug=False,
        enable_asserts=True,
        num_devices=N_CORES,
    )
    xT = nc.dram_tensor("xT", [DIN, tok], F32, kind="ExternalInput")
    w_enc = nc.dram_tensor("w_enc", [DIN, H], F32, kind="ExternalInput")
    b_enc = nc.dram_tensor("b_enc", [128, 2], F32, kind="ExternalInput")
    w_top = nc.dram_tensor("w_top", [K, 2, 128, H], F32, kind="ExternalInput")
    w_bot = nc.dram_tensor("w_bot", [K, 2, 128, H], F32, kind="ExternalInput")
    b_h = nc.dram_tensor("b_h", [128, K * 2], F32, kind="ExternalInput")
    w_dec = nc.dram_tensor("w_dec", [2, 128, DOUT], F32, kind="ExternalInput")
    b_dec = nc.dram_tensor("b_dec", [DOUT, 1], F32, kind="ExternalInput")
    sel = nc.dram_tensor("sel", [ST_EX, ST], F32, kind="ExternalInput")
    y = nc.dram_tensor("y", [DOUT, tok], F32, kind="ExternalOutput")

    with tile.TileContext(nc) as tc:
        with (
            tc.tile_pool(name="wpool", bufs=1) as wpool,
            tc.tile_pool(name="xpool", bufs=4) as xpool,
            tc.tile_pool(name="hpool", bufs=4) as hpool,
            tc.tile_pool(name="opool", bufs=2) as opool,
            tc.tile_pool(name="cpool", bufs=4) as cpool,
            tc.tile_pool(name="psmm", bufs=3, space=bass.MemorySpace.PSUM) as psmm,
            tc.tile_pool(name="psdec", bufs=2, space=bass.MemorySpace.PSUM) as psdec,
        ):
            # --- weights: casting DMAs (f32 -> fp16), resident for the run ---
            wenc_sb = wpool.tile([DIN, H], F16)
            nc.gpsimd.dma_start(wenc_sb[:], w_enc[:])
            benc_sb = wpool.tile([128, 2], F32)
            nc.sync.dma_start(benc_sb[:], b_enc[:])
            wtop_sb = wpool.tile([128, K * 2 * H], F16)
            wbot_sb = wpool.tile([128, K * 2 * H], F16)
            for k in range(K):
                for kc in range(2):
                    off = (k * 2 + kc) * H
                    nc.gpsimd.dma_start(wtop_sb[:, off : off + H], w_top[k, kc])
                    nc.gpsimd.dma_start(wbot_sb[:, off : off + H], w_bot[k, kc])
            bh_sb = wpool.tile([128, K * 2], F32)
            nc.sync.dma_start(bh_sb[:], b_h[:])
            wdec_sb = wpool.tile([128, 2 * DOUT], F16)
            for kc in range(2):
                nc.gpsimd.dma_start(wdec_sb[:, kc * DOUT : (kc + 1) * DOUT], w_dec[kc])
            bdec_sb = wpool.tile([DOUT, 1], F32)
            nc.sync.dma_start(bdec_sb[:], b_dec[:])
            sel_sb = wpool.tile([ST_EX, ST], F16)
            nc.gpsimd.dma_start(sel_sb[:], sel[:])

            ILV = 4  # supertiles emitted in interleaved phase groups

            def psum_view(ps, g):
                return ps[:].rearrange("p (g b) -> p g b", b=BANK)[:, 0:g, 0:SUB]

            def reduce_seg(hout, lo, hi, c_out, m, g0, g1):
                seg = hout[:, lo:hi].rearrange("p (b a) -> p b a", a=A)
                with nc.allow_low_precision(
                    reason="fp16 out rounding; accumulation is fp32"
                ):
                    nc.vector.reduce_sum(
                        c_out[:, m, g0 * SUB_EX : g1 * SUB_EX],
                        seg,
                        axis=mybir.AxisListType.X,
                    )

            def make_state(s):
                xt = xpool.tile([DIN, ST], F16, tag="xt", name=f"xt_{s}")
                nc.gpsimd.dma_start(xt[:], xT[:, s * ST : (s + 1) * ST])
                hA = [
                    hpool.tile([128, ST], F16, tag=f"hA{m}", name=f"hA{m}_{s}")
                    for m in range(2)
                ]
                hB = [
                    hpool.tile([128, ST], F16, tag=f"hB{m}", name=f"hB{m}_{s}")
                    for m in range(2)
                ]
                c_ts = [
                    cpool.tile([128, 2, ST_EX], F16, tag=f"c{k}", name=f"c{k}_{s}")
                    for k in range(K)
                ]
                return {"s": s, "xt": xt, "hA": hA, "hB": hB, "c": c_ts}

            def enc_phase(st):
                s, xt, hA = st["s"], st["xt"], st["hA"]
                for m in range(2):
                    for grp in TANH_GROUPS:
                        g = len(grp)
                        ps = psmm.tile(
                            [128, 2 * BANK], F32, tag="ps", name=f"pse_{s}_{m}_{grp[0]}"
                        )
                        for j, n in enumerate(grp):
                            nc.tensor.matmul(
                                ps[:, j * BANK : j * BANK + SUB],
                                wenc_sb[:, m * 128 : (m + 1) * 128],
                                xt[:, n * SUB : (n + 1) * SUB],
                                start=True,
                                stop=True,
                            )
                        lo, hi = grp[0] * SUB, (grp[-1] + 1) * SUB
                        hv = hA[m][:, lo:hi].rearrange("p (g b) -> p g b", b=SUB)
                        nc.scalar.activation(
                            hv, psum_view(ps, g), Tanh, bias=benc_sb[:, m : m + 1]
                        )
                        reduce_seg(hA[m], lo, hi, st["c"][0], m, grp[0], grp[-1] + 1)

            def comm_phase(st, k):
                s, c_t = st["s"], st["c"][k]
                hcur = st["hA"] if k == 0 else st["hB"]
                hnxt = st["hB"] if k == 0 else st["hA"]
                # cwT[ex, feat] = c.T @ W_bot (c is already [feat, ex] = lhsT)
                pcw = psdec.tile([ST_EX, H], F32, tag="pd", name=f"pcw_{s}_{k}")
                for kc in range(2):
                    off = (k * 2 + kc) * H
                    nc.tensor.matmul(
                        pcw[:],
                        c_t[:, kc, :],
                        wbot_sb[:, off : off + H],
                        start=(kc == 0),
                        stop=(kc == 1),
                    )
                cwT_sb = cpool.tile([ST_EX, H], F16, tag="cwT", name=f"cwT_{s}_{k}")
                nc.vector.tensor_copy(cwT_sb[:], pcw[:])
                # h' = tanh(W_top.T @ h + cw(bcast via selector matmul) + b_h)
                for m in range(2):
                    for grp in TANH_GROUPS:
                        g = len(grp)
                        ps = psmm.tile(
                            [128, 2 * BANK], F32, tag="ps",
                            name=f"psc_{s}_{k}_{m}_{grp[0]}",
                        )
                        for kc in range(2):
                            off = (k * 2 + kc) * H + m * 128
                            for j, n in enumerate(grp):
                                nc.tensor.matmul(
                                    ps[:, j * BANK : j * BANK + SUB],
                                    wtop_sb[:, off : off + 128],
                                    hcur[kc][:, n * SUB : (n + 1) * SUB],
                                    start=(kc == 0),
                                    stop=False,
                                )
                        for j, n in enumerate(grp):
                            nc.tensor.matmul(
                                ps[:, j * BANK : j * BANK + SUB],
                                cwT_sb[:, m * 128 : (m + 1) * 128],
                                sel_sb[:, n * SUB : (n + 1) * SUB],
                                start=False,
                                stop=True,
                            )
                        lo, hi = grp[0] * SUB, (grp[-1] + 1) * SUB
                        hv = hnxt[m][:, lo:hi].rearrange("p (g b) -> p g b", b=SUB)
                        nc.scalar.activation(
                            hv, psum_view(ps, g), Tanh,
                            bias=bh_sb[:, k * 2 + m : k * 2 + m + 1],
                        )
                        if k + 1 < K:
                            reduce_seg(
                                hnxt[m], lo, hi, st["c"][k + 1], m,
                                grp[0], grp[-1] + 1,
                            )

            def dec_phase(st):
                s = st["s"]
                hcur = st["hA"] if K % 2 == 0 else st["hB"]
                out_t = opool.tile([DOUT, ST], F32, tag="out", name=f"out_{s}")
                for n in range(NSUB):
                    pd = psdec.tile([DOUT, SUB], F32, tag="pd", name=f"pd_{s}_{n}")
                    for kc in range(2):
                        nc.tensor.matmul(
                            pd[:],
                            wdec_sb[:, kc * DOUT : (kc + 1) * DOUT],
                            hcur[kc][:, n * SUB : (n + 1) * SUB],
                            start=(kc == 0),
                            stop=(kc == 1),
                        )
                    nc.vector.tensor_scalar_add(
                        out_t[:, n * SUB : (n + 1) * SUB], pd[:], bdec_sb[:, 0:1]
                    )
                nc.sync.dma_start(y[:, s * ST : (s + 1) * ST], out_t[:])

            assert n_supertiles % ILV == 0 or n_supertiles < ILV
            step = min(ILV, n_supertiles)
            groups = [
                list(range(s0, s0 + step))
                for s0 in range(0, n_supertiles, step)
            ]
            sts = [make_state(s) for s in groups[0]]
            for st in sts:
                enc_phase(st)
            for gi, grp in enumerate(groups):
                for k in range(K):
                    for st in sts:
                        comm_phase(st, k)
                if gi + 1 < len(groups):
                    # prefetch next group's inputs, then hide each dec tail
                    # under the next group's encoder matmuls
                    nxt = [make_state(s) for s in groups[gi + 1]]
                    for st, nst in zip(sts, nxt):
                        dec_phase(st)
                        enc_phase(nst)
                    sts = nxt
                else:
                    for st in sts:
                        dec_phase(st)

    nc.compile()
    return nc


def host_inputs(x, W_enc, b_enc, W_h, b_h, W_dec, b_dec, n_cores=N_CORES, bs=BS):
    """Shard x over cores (pre-transposed to [DIN, tok]); replicate weights."""
    x = np.asarray(x, np.float32)
    common = {
        "w_enc": np.ascontiguousarray(np.asarray(W_enc, np.float32)),
        "b_enc": np.ascontiguousarray(
            np.asarray(b_enc, np.float32).reshape(2, 128).T
        ),
        "w_top": np.ascontiguousarray(
            np.asarray(W_h, np.float32)[:, :H, :].reshape(K, 2, 128, H)
        ),
        "w_bot": np.ascontiguousarray(
            (np.asarray(W_h, np.float32)[:, H:, :] / A).reshape(K, 2, 128, H)
        ),
        "b_h": np.ascontiguousarray(
            np.asarray(b_h, np.float32).reshape(K, 2, 128).transpose(2, 0, 1).reshape(128, K * 2)
        ),
        "w_dec": np.ascontiguousarray(
            np.asarray(W_dec, np.float32).reshape(2, 128, DOUT)
        ),
        "b_dec": np.ascontiguousarray(np.asarray(b_dec, np.float32).reshape(DOUT, 1)),
        "sel": np.ascontiguousarray(
            np.repeat(np.eye(ST_EX, dtype=np.float32), A, axis=1)
        ),
    }
    in_maps = []
    for i in range(n_cores):
        shard = x[i * bs : (i + 1) * bs].reshape(bs * A, DIN)
        in_maps.append({**common, "xT": np.ascontiguousarray(shard.T)})
    return in_maps


_NC_CACHE = None


def _get_nc():
    global _NC_CACHE
    if _NC_CACHE is None:
        _NC_CACHE = build_nc()
    return _NC_CACHE


def kernel(x, W_enc, b_enc, W_h, b_h, W_dec, b_dec, _run_kwargs=None):
    in_maps = host_inputs(x, W_enc, b_enc, W_h, b_h, W_dec, b_dec)
    nc = _get_nc()
    res = run_bass_kernel_spmd(nc, in_maps, list(range(N_CORES)), **(_run_kwargs or {}))
    outs = [
        res.results[i]["y"].T.reshape(BS, A, DOUT).astype(np.float32)
        for i in range(N_CORES)
    ]
    full = np.concatenate(outs, axis=0)
    if _run_kwargs:
        kernel.last_results = res
    return full


# revision 15
# speedup vs baseline: 1.6790x; 1.0146x over previous
"""CommNet (B=4096, A=50, DIN=128, H=256, DOUT=64, K=2) on 8 TRN2 NeuronCores.

Data-parallel over the batch axis: 512 examples (25600 agent-tokens) per core,
weights replicated. On-chip layout is feature-major ([feature, token]) so every
layer's contraction dim sits on SBUF partitions; the host pre-transposes each
x shard once (numpy) so no on-chip transposes are needed.

Per comm step the concat [h, c] @ W is split as h @ W_top + c @ W_bot with the
1/50 agent-mean folded into W_bot on the host. The per-example c @ W_bot result
(computed transposed, with c as the stationary matmul operand) is broadcast
back over agents by a third accumulating matmul against a constant 0/1
selector, so the whole comm step stays on the PE and lands in one PSUM tile.

Matmul operands are fp16 (1 cyc/row, fast weight load; PSUM accumulation is
fp32). tanh runs on ScalarE over three bank-aligned PSUM sub-tiles per
ACTIVATE (N=1200) to amortize the per-op ~352-cycle overhead. VectorE does the
per-example agent-sum reductions (fp16 2x mode) and the decoder bias-add;
GpSimd only drives the casting DMAs.
"""

import numpy as np

import concourse.bacc as bacc
import concourse.bass as bass
import concourse.tile as tile
from concourse import mybir
from concourse.bass_utils import run_bass_kernel_spmd

N_CORES = 8
B, A, DIN, H, DOUT, K = 4096, 50, 128, 256, 64, 2
BS = B // N_CORES          # examples per core
TOK = BS * A               # tokens per core
ST_EX = 64                 # examples per supertile
ST = ST_EX * A             # 3200 tokens per supertile
SUB_EX = 8                 # examples per matmul sub-tile
SUB = SUB_EX * A           # 400 tokens (PSUM bank limit: N <= 512 fp32 accum)
NSUB = ST // SUB           # 8
BANK = 512                 # fp32 elems per PSUM bank
RSUB = 1600                # tokens per DVE reduce op (32 examples)

F32 = mybir.dt.float32
F16 = mybir.dt.float16
Tanh = mybir.ActivationFunctionType.Tanh

# tanh batching: groups of matmul sub-tiles sharing one PSUM tile + ACTIVATE
TANH_GROUPS = [(0, 1), (2, 3), (4, 5), (6, 7)]


def build_nc(n_supertiles=BS // ST_EX):
    tok = n_supertiles * ST
    nc = bacc.Bacc(
        "TRN2",
        target_bir_lowering=False,
        debug=False,
        enable_asserts=True,
        num_devices=N_CORES,
    )
    xT = nc.dram_tensor("xT", [DIN, tok], F32, kind="ExternalInput")
    w_enc = nc.dram_tensor("w_enc", [DIN, H], F32, kind="ExternalInput")
    b_enc = nc.dram_tensor("b_enc", [128, 2], F32, kind="ExternalInput")
    w_top = nc.dram_tensor("w_top", [K, 2, 128, H], F32, kind="ExternalInput")
    w_bot = nc.dram_tensor("w_bot", [K, 2, 128, H], F32, kind="ExternalInput")
    b_h = nc.dram_tensor("b_h", [128, K * 2], F32, kind="ExternalInput")
    w_dec = nc.dram_tensor("w_dec", [2, 128, DOUT], F32, kind="ExternalInput")
    b_dec = nc.dram_tensor("b_dec", [DOUT, 1], F32, kind="ExternalInput")
    sel = nc.dram_tensor("sel", [ST_EX, ST], F32, kind="ExternalInput")
    y = nc.dram_tensor("y", [DOUT, tok], F32, kind="ExternalOutput")

    with tile.TileContext(nc) as tc:
        with (
            tc.tile_pool(name="wpool", bufs=1) as wpool,
            tc.tile_pool(name="xpool", bufs=4) as xpool,
            tc.tile_pool(name="hpool", bufs=4) as hpool,
            tc.tile_pool(name="opool", bufs=2) as opool,
            tc.tile_pool(name="cpool", bufs=4) as cpool,
            tc.tile_pool(name="psmm", bufs=3, space=bass.MemorySpace.PSUM) as psmm,
            tc.tile_pool(name="psdec", bufs=2, space=bass.MemorySpace.PSUM) as psdec,
        ):
            # --- weights: casting DMAs (f32 -> fp16), resident for the run ---
            wenc_sb = wpool.tile([DIN, H], F16)
            nc.gpsimd.dma_start(wenc_sb[:], w_enc[:])
            benc_sb = wpool.tile([128, 2], F32)
            nc.sync.dma_start(benc_sb[:], b_enc[:])
            wtop_sb = wpool.tile([128, K * 2 * H], F16)
            wbot_sb = wpool.tile([128, K * 2 * H], F16)
            bh_sb = wpool.tile([128, K * 2], F32)
            nc.sync.dma_start(bh_sb[:], b_h[:])
            wdec_sb = wpool.tile([128, 2 * DOUT], F16)
            bdec_sb = wpool.tile([DOUT, 1], F32)
            nc.sync.dma_start(bdec_sb[:], b_dec[:])
            sel_sb = wpool.tile([ST_EX, ST], F16)

            def load_bulk_weights():
                for k in range(K):
                    for kc in range(2):
                        off = (k * 2 + kc) * H
                        nc.gpsimd.dma_start(wtop_sb[:, off : off + H], w_top[k, kc])
                        nc.gpsimd.dma_start(wbot_sb[:, off : off + H], w_bot[k, kc])
                for kc in range(2):
                    nc.gpsimd.dma_start(
                        wdec_sb[:, kc * DOUT : (kc + 1) * DOUT], w_dec[kc]
                    )
                nc.gpsimd.dma_start(sel_sb[:], sel[:])

            ILV = 4  # supertiles emitted in interleaved phase groups

            def psum_view(ps, g):
                return ps[:].rearrange("p (g b) -> p g b", b=BANK)[:, 0:g, 0:SUB]

            def reduce_seg(hout, lo, hi, c_out, m, g0, g1):
                seg = hout[:, lo:hi].rearrange("p (b a) -> p b a", a=A)
                with nc.allow_low_precision(
                    reason="fp16 out rounding; accumulation is fp32"
                ):
                    nc.vector.reduce_sum(
                        c_out[:, m, g0 * SUB_EX : g1 * SUB_EX],
                        seg,
                        axis=mybir.AxisListType.X,
                    )

            def make_state(s):
                xt = xpool.tile([DIN, ST], F16, tag="xt", name=f"xt_{s}")
                for c0 in range(0, ST, 2 * SUB):
                    nc.gpsimd.dma_start(
                        xt[:, c0 : c0 + 2 * SUB],
                        xT[:, s * ST + c0 : s * ST + c0 + 2 * SUB],
                    )
                hA = [
                    hpool.tile([128, ST], F16, tag=f"hA{m}", name=f"hA{m}_{s}")
                    for m in range(2)
                ]
                hB = [
                    hpool.tile([128, ST], F16, tag=f"hB{m}", name=f"hB{m}_{s}")
                    for m in range(2)
                ]
                c_ts = [
                    cpool.tile([128, 2, ST_EX], F16, tag=f"c{k}", name=f"c{k}_{s}")
                    for k in range(K)
                ]
                return {"s": s, "xt": xt, "hA": hA, "hB": hB, "c": c_ts}

            def enc_phase(st):
                s, xt, hA = st["s"], st["xt"], st["hA"]
                for m in range(2):
                    for grp in TANH_GROUPS:
                        g = len(grp)
                        ps = psmm.tile(
                            [128, 2 * BANK], F32, tag="ps", name=f"pse_{s}_{m}_{grp[0]}"
                        )
                        for j, n in enumerate(grp):
                            nc.tensor.matmul(
                                ps[:, j * BANK : j * BANK + SUB],
                                wenc_sb[:, m * 128 : (m + 1) * 128],
                                xt[:, n * SUB : (n + 1) * SUB],
                                start=True,
                                stop=True,
                            )
                        lo, hi = grp[0] * SUB, (grp[-1] + 1) * SUB
                        hv = hA[m][:, lo:hi].rearrange("p (g b) -> p g b", b=SUB)
                        nc.scalar.activation(
                            hv, psum_view(ps, g), Tanh, bias=benc_sb[:, m : m + 1]
                        )
                        reduce_seg(hA[m], lo, hi, st["c"][0], m, grp[0], grp[-1] + 1)

            def comm_phase(st, k):
                s, c_t = st["s"], st["c"][k]
                hcur = st["hA"] if k == 0 else st["hB"]
                hnxt = st["hB"] if k == 0 else st["hA"]
                # cwT[ex, feat] = c.T @ W_bot (c is already [feat, ex] = lhsT)
                pcw = psdec.tile([ST_EX, H], F32, tag="pd", name=f"pcw_{s}_{k}")
                for kc in range(2):
                    off = (k * 2 + kc) * H
                    nc.tensor.matmul(
                        pcw[:],
                        c_t[:, kc, :],
                        wbot_sb[:, off : off + H],
                        start=(kc == 0),
                        stop=(kc == 1),
                    )
                cwT_sb = cpool.tile([ST_EX, H], F16, tag="cwT", name=f"cwT_{s}_{k}")
                nc.vector.tensor_copy(cwT_sb[:], pcw[:])
                # h' = tanh(W_top.T @ h + cw(bcast via selector matmul) + b_h)
                for m in range(2):
                    for grp in TANH_GROUPS:
                        g = len(grp)
                        ps = psmm.tile(
                            [128, 2 * BANK], F32, tag="ps",
                            name=f"psc_{s}_{k}_{m}_{grp[0]}",
                        )
                        for kc in range(2):
                            off = (k * 2 + kc) * H + m * 128
                            for j, n in enumerate(grp):
                                nc.tensor.matmul(
                                    ps[:, j * BANK : j * BANK + SUB],
                                    wtop_sb[:, off : off + 128],
                                    hcur[kc][:, n * SUB : (n + 1) * SUB],
                                    start=(kc == 0),
                                    stop=False,
                                )
                        for j, n in enumerate(grp):
                            nc.tensor.matmul(
                                ps[:, j * BANK : j * BANK + SUB],
                                cwT_sb[:, m * 128 : (m + 1) * 128],
                                sel_sb[:, n * SUB : (n + 1) * SUB],
                                start=False,
                                stop=True,
                            )
                        lo, hi = grp[0] * SUB, (grp[-1] + 1) * SUB
                        hv = hnxt[m][:, lo:hi].rearrange("p (g b) -> p g b", b=SUB)
                        nc.scalar.activation(
                            hv, psum_view(ps, g), Tanh,
                            bias=bh_sb[:, k * 2 + m : k * 2 + m + 1],
                        )
                        if k + 1 < K:
                            reduce_seg(
                                hnxt[m], lo, hi, st["c"][k + 1], m,
                                grp[0], grp[-1] + 1,
                            )

            def dec_phase(st):
                s = st["s"]
                hcur = st["hA"] if K % 2 == 0 else st["hB"]
                out_t = opool.tile([DOUT, ST], F32, tag="out", name=f"out_{s}")
                for n in range(NSUB):
                    pd = psdec.tile([DOUT, SUB], F32, tag="pd", name=f"pd_{s}_{n}")
                    for kc in range(2):
                        nc.tensor.matmul(
                            pd[:],
                            wdec_sb[:, kc * DOUT : (kc + 1) * DOUT],
                            hcur[kc][:, n * SUB : (n + 1) * SUB],
                            start=(kc == 0),
                            stop=(kc == 1),
                        )
                    nc.vector.tensor_scalar_add(
                        out_t[:, n * SUB : (n + 1) * SUB], pd[:], bdec_sb[:, 0:1]
                    )
                nc.sync.dma_start(y[:, s * ST : (s + 1) * ST], out_t[:])

            assert n_supertiles % ILV == 0 or n_supertiles < ILV
            step = min(ILV, n_supertiles)
            groups = [
                list(range(s0, s0 + step))
                for s0 in range(0, n_supertiles, step)
            ]
            sts = [make_state(s) for s in groups[0]]
            load_bulk_weights()
            for st in sts:
                enc_phase(st)
            for gi, grp in enumerate(groups):
                for k in range(K):
                    for st in sts:
                        comm_phase(st, k)
                if gi + 1 < len(groups):
                    # prefetch next group's inputs, then hide each dec tail
                    # under the next group's encoder matmuls
                    nxt = [make_state(s) for s in groups[gi + 1]]
                    for st, nst in zip(sts, nxt):
                        dec_phase(st)
                        enc_phase(nst)
                    sts = nxt
                else:
                    for st in sts:
                        dec_phase(st)

    nc.compile()
    return nc


def host_inputs(x, W_enc, b_enc, W_h, b_h, W_dec, b_dec, n_cores=N_CORES, bs=BS):
    """Shard x over cores (pre-transposed to [DIN, tok]); replicate weights."""
    x = np.asarray(x, np.float32)
    common = {
        "w_enc": np.ascontiguousarray(np.asarray(W_enc, np.float32)),
        "b_enc": np.ascontiguousarray(
            np.asarray(b_enc, np.float32).reshape(2, 128).T
        ),
        "w_top": np.ascontiguousarray(
            np.asarray(W_h, np.float32)[:, :H, :].reshape(K, 2, 128, H)
        ),
        "w_bot": np.ascontiguousarray(
            (np.asarray(W_h, np.float32)[:, H:, :] / A).reshape(K, 2, 128, H)
        ),
        "b_h": np.ascontiguousarray(
            np.asarray(b_h, np.float32).reshape(K, 2, 128).transpose(2, 0, 1).reshape(128, K * 2)
        ),
        "w_dec": np.ascontiguousarray(
            np.asarray(W_dec, np.float32).reshape(2, 128, DOUT)
        ),
        "b_dec": np.ascontiguousarray(np.asarray(b_dec, np.float32).reshape(DOUT, 1)),
        "sel": np.ascontiguousarray(
            np.repeat(np.eye(ST_EX, dtype=np.float32), A, axis=1)
        ),
    }
    in_maps = []
    for i in range(n_cores):
        shard = x[i * bs : (i + 1) * bs].reshape(bs * A, DIN)
        in_maps.append({**common, "xT": np.ascontiguousarray(shard.T)})
    return in_maps


_NC_CACHE = None


def _get_nc():
    global _NC_CACHE
    if _NC_CACHE is None:
        _NC_CACHE = build_nc()
    return _NC_CACHE


def kernel(x, W_enc, b_enc, W_h, b_h, W_dec, b_dec, _run_kwargs=None):
    in_maps = host_inputs(x, W_enc, b_enc, W_h, b_h, W_dec, b_dec)
    nc = _get_nc()
    res = run_bass_kernel_spmd(nc, in_maps, list(range(N_CORES)), **(_run_kwargs or {}))
    outs = [
        res.results[i]["y"].T.reshape(BS, A, DOUT).astype(np.float32)
        for i in range(N_CORES)
    ]
    full = np.concatenate(outs, axis=0)
    if _run_kwargs:
        kernel.last_results = res
    return full


# revision 18
# speedup vs baseline: 1.6961x; 1.0102x over previous
"""CommNet (B=4096, A=50, DIN=128, H=256, DOUT=64, K=2) on 8 TRN2 NeuronCores.

Data-parallel over the batch axis: 512 examples (25600 agent-tokens) per core,
weights replicated. On-chip layout is feature-major ([feature, token]) so every
layer's contraction dim sits on SBUF partitions; the host pre-transposes each
x shard once (numpy) so no on-chip transposes are needed.

Per comm step the concat [h, c] @ W is split as h @ W_top + c @ W_bot with the
1/50 agent-mean folded into W_bot on the host. The per-example c @ W_bot result
(computed transposed, with c as the stationary matmul operand) is broadcast
back over agents by a third accumulating matmul against a constant 0/1
selector, so the whole comm step stays on the PE and lands in one PSUM tile.

Matmul operands are fp16 (1 cyc/row, fast weight load; PSUM accumulation is
fp32). tanh runs on ScalarE over two bank-aligned PSUM sub-tiles per ACTIVATE
(N=800) to amortize the per-op ~352-cycle overhead; agent-sum reductions are
emitted incrementally right after each tanh group so the next layer's cwT
chain never stalls the PE. Supertiles are emitted in 4-way interleaved phase
groups (enc x4 -> comm0 x4 -> comm1 x4 -> dec x4, with the next group's
encoder hidden under the current group's decoder) to keep the PE dense and the
HAM clock-gate warm. Measured: ~289 us HW exec, rel err ~5.7e-4 vs the fp32
reference.
"""

import numpy as np

import concourse.bacc as bacc
import concourse.bass as bass
import concourse.tile as tile
from concourse import mybir
from concourse.bass_utils import run_bass_kernel_spmd

N_CORES = 8
B, A, DIN, H, DOUT, K = 4096, 50, 128, 256, 64, 2
BS = B // N_CORES          # examples per core
TOK = BS * A               # tokens per core
ST_EX = 64                 # examples per supertile
ST = ST_EX * A             # 3200 tokens per supertile
SUB_EX = 8                 # examples per matmul sub-tile
SUB = SUB_EX * A           # 400 tokens (PSUM bank limit: N <= 512 fp32 accum)
NSUB = ST // SUB           # 8
BANK = 512                 # fp32 elems per PSUM bank
RSUB = 1600                # tokens per DVE reduce op (32 examples)

F32 = mybir.dt.float32
F16 = mybir.dt.float16
Tanh = mybir.ActivationFunctionType.Tanh

# tanh batching: groups of matmul sub-tiles sharing one PSUM tile + ACTIVATE
TANH_GROUPS = [(0, 1), (2, 3), (4, 5), (6, 7)]


def build_nc(n_supertiles=BS // ST_EX):
    tok = n_supertiles * ST
    nc = bacc.Bacc(
        "TRN2",
        target_bir_lowering=False,
        debug=False,
        enable_asserts=True,
        num_devices=N_CORES,
    )
    xT = nc.dram_tensor("xT", [DIN, tok], F32, kind="ExternalInput")
    w_enc = nc.dram_tensor("w_enc", [DIN, H], F32, kind="ExternalInput")
    b_enc = nc.dram_tensor("b_enc", [128, 2], F32, kind="ExternalInput")
    w_top = nc.dram_tensor("w_top", [K, 2, 128, H], F32, kind="ExternalInput")
    w_bot = nc.dram_tensor("w_bot", [K, 2, 128, H], F32, kind="ExternalInput")
    b_h = nc.dram_tensor("b_h", [128, K * 2], F32, kind="ExternalInput")
    w_dec = nc.dram_tensor("w_dec", [2, 128, DOUT], F32, kind="ExternalInput")
    b_dec = nc.dram_tensor("b_dec", [DOUT, 1], F32, kind="ExternalInput")
    sel = nc.dram_tensor("sel", [ST_EX, ST], F32, kind="ExternalInput")
    y = nc.dram_tensor("y", [DOUT, tok], F32, kind="ExternalOutput")

    with tile.TileContext(nc) as tc:
        with (
            tc.tile_pool(name="wpool", bufs=1) as wpool,
            tc.tile_pool(name="xpool", bufs=4) as xpool,
            tc.tile_pool(name="hpool", bufs=4) as hpool,
            tc.tile_pool(name="opool", bufs=2) as opool,
            tc.tile_pool(name="cpool", bufs=4) as cpool,
            tc.tile_pool(name="psmm", bufs=3, space=bass.MemorySpace.PSUM) as psmm,
            tc.tile_pool(name="psdec", bufs=2, space=bass.MemorySpace.PSUM) as psdec,
        ):
            # --- weights: casting DMAs (f32 -> fp16), resident for the run ---
            wenc_sb = wpool.tile([DIN, H], F16)
            nc.gpsimd.dma_start(wenc_sb[:], w_enc[:])
            benc_sb = wpool.tile([128, 2], F32)
            nc.sync.dma_start(benc_sb[:], b_enc[:])
            wtop_sb = wpool.tile([128, K * 2 * H], F16)
            wbot_sb = wpool.tile([128, K * 2 * H], F16)
            bh_sb = wpool.tile([128, K * 2], F32)
            nc.sync.dma_start(bh_sb[:], b_h[:])
            wdec_sb = wpool.tile([128, 2 * DOUT], F16)
            bdec_sb = wpool.tile([DOUT, 1], F32)
            nc.sync.dma_start(bdec_sb[:], b_dec[:])
            sel_sb = wpool.tile([ST_EX, ST], F16)

            def load_bulk_weights():
                for k in range(K):
                    for kc in range(2):
                        off = (k * 2 + kc) * H
                        nc.gpsimd.dma_start(wtop_sb[:, off : off + H], w_top[k, kc])
                        nc.gpsimd.dma_start(wbot_sb[:, off : off + H], w_bot[k, kc])
                for kc in range(2):
                    nc.gpsimd.dma_start(
                        wdec_sb[:, kc * DOUT : (kc + 1) * DOUT], w_dec[kc]
                    )
                nc.gpsimd.dma_start(sel_sb[:], sel[:])

            ILV = 4  # supertiles emitted in interleaved phase groups

            def psum_view(ps, g):
                return ps[:].rearrange("p (g b) -> p g b", b=BANK)[:, 0:g, 0:SUB]

            def reduce_seg(hout, lo, hi, c_out, m, g0, g1):
                seg = hout[:, lo:hi].rearrange("p (b a) -> p b a", a=A)
                with nc.allow_low_precision(
                    reason="fp16 out rounding; accumulation is fp32"
                ):
                    nc.vector.reduce_sum(
                        c_out[:, m, g0 * SUB_EX : g1 * SUB_EX],
                        seg,
                        axis=mybir.AxisListType.X,
                    )

            def make_state(s):
                xt = xpool.tile([DIN, ST], F16, tag="xt", name=f"xt_{s}")
                for c0 in range(0, ST, 2 * SUB):
                    nc.gpsimd.dma_start(
                        xt[:, c0 : c0 + 2 * SUB],
                        xT[:, s * ST + c0 : s * ST + c0 + 2 * SUB],
                    )
                hA = [
                    hpool.tile([128, ST], F16, tag=f"hA{m}", name=f"hA{m}_{s}")
                    for m in range(2)
                ]
                hB = [
                    hpool.tile([128, ST], F16, tag=f"hB{m}", name=f"hB{m}_{s}")
                    for m in range(2)
                ]
                c_ts = [
                    cpool.tile([128, 2, ST_EX], F16, tag=f"c{k}", name=f"c{k}_{s}")
                    for k in range(K)
                ]
                return {"s": s, "xt": xt, "hA": hA, "hB": hB, "c": c_ts}

            def enc_group(st, m, grp):
                s, xt, hA = st["s"], st["xt"], st["hA"]
                g = len(grp)
                ps = psmm.tile(
                    [128, 2 * BANK], F32, tag="ps", name=f"pse_{s}_{m}_{grp[0]}"
                )
                for j, n in enumerate(grp):
                    nc.tensor.matmul(
                        ps[:, j * BANK : j * BANK + SUB],
                        wenc_sb[:, m * 128 : (m + 1) * 128],
                        xt[:, n * SUB : (n + 1) * SUB],
                        start=True,
                        stop=True,
                    )
                lo, hi = grp[0] * SUB, (grp[-1] + 1) * SUB
                hv = hA[m][:, lo:hi].rearrange("p (g b) -> p g b", b=SUB)
                nc.scalar.activation(
                    hv, psum_view(ps, g), Tanh, bias=benc_sb[:, m : m + 1]
                )
                reduce_seg(hA[m], lo, hi, st["c"][0], m, grp[0], grp[-1] + 1)

            def enc_phase(st):
                for m in range(2):
                    for grp in TANH_GROUPS:
                        enc_group(st, m, grp)

            def comm_phase(st, k):
                s, c_t = st["s"], st["c"][k]
                hcur = st["hA"] if k == 0 else st["hB"]
                hnxt = st["hB"] if k == 0 else st["hA"]
                # cwT[ex, feat] = c.T @ W_bot (c is already [feat, ex] = lhsT)
                pcw = psdec.tile([ST_EX, H], F32, tag="pd", name=f"pcw_{s}_{k}")
                for kc in range(2):
                    off = (k * 2 + kc) * H
                    nc.tensor.matmul(
                        pcw[:],
                        c_t[:, kc, :],
                        wbot_sb[:, off : off + H],
                        start=(kc == 0),
                        stop=(kc == 1),
                    )
                cwT_sb = cpool.tile([ST_EX, H], F16, tag="cwT", name=f"cwT_{s}_{k}")
                nc.vector.tensor_copy(cwT_sb[:], pcw[:])
                # h' = tanh(W_top.T @ h + cw(bcast via selector matmul) + b_h)
                for m in range(2):
                    for grp in TANH_GROUPS:
                        g = len(grp)
                        ps = psmm.tile(
                            [128, 2 * BANK], F32, tag="ps",
                            name=f"psc_{s}_{k}_{m}_{grp[0]}",
                        )
                        for kc in range(2):
                            off = (k * 2 + kc) * H + m * 128
                            for j, n in enumerate(grp):
                                nc.tensor.matmul(
                                    ps[:, j * BANK : j * BANK + SUB],
                                    wtop_sb[:, off : off + 128],
                                    hcur[kc][:, n * SUB : (n + 1) * SUB],
                                    start=(kc == 0),
                                    stop=False,
                                )
                        for j, n in enumerate(grp):
                            nc.tensor.matmul(
                                ps[:, j * BANK : j * BANK + SUB],
                                cwT_sb[:, m * 128 : (m + 1) * 128],
                                sel_sb[:, n * SUB : (n + 1) * SUB],
                                start=False,
                                stop=True,
                            )
                        lo, hi = grp[0] * SUB, (grp[-1] + 1) * SUB
                        hv = hnxt[m][:, lo:hi].rearrange("p (g b) -> p g b", b=SUB)
                        nc.scalar.activation(
                            hv, psum_view(ps, g), Tanh,
                            bias=bh_sb[:, k * 2 + m : k * 2 + m + 1],
                        )
                        if k + 1 < K:
                            reduce_seg(
                                hnxt[m], lo, hi, st["c"][k + 1], m,
                                grp[0], grp[-1] + 1,
                            )

            def dec_phase(st):
                s = st["s"]
                hcur = st["hA"] if K % 2 == 0 else st["hB"]
                out_t = opool.tile([DOUT, ST], F32, tag="out", name=f"out_{s}")
                for n in range(NSUB):
                    pd = psdec.tile([DOUT, SUB], F32, tag="pd", name=f"pd_{s}_{n}")
                    for kc in range(2):
                        nc.tensor.matmul(
                            pd[:],
                            wdec_sb[:, kc * DOUT : (kc + 1) * DOUT],
                            hcur[kc][:, n * SUB : (n + 1) * SUB],
                            start=(kc == 0),
                            stop=(kc == 1),
                        )
                    nc.vector.tensor_scalar_add(
                        out_t[:, n * SUB : (n + 1) * SUB], pd[:], bdec_sb[:, 0:1]
                    )
                nc.sync.dma_start(y[:, s * ST : (s + 1) * ST], out_t[:])

            assert n_supertiles % ILV == 0 or n_supertiles < ILV
            step = min(ILV, n_supertiles)
            groups = [
                list(range(s0, s0 + step))
                for s0 in range(0, n_supertiles, step)
            ]
            sts = [make_state(s) for s in groups[0]]
            load_bulk_weights()
            for st in sts:
                enc_phase(st)
            for gi, grp in enumerate(groups):
                for k in range(K):
                    for st in sts:
                        comm_phase(st, k)
                if gi + 1 < len(groups):
                    # prefetch next group's inputs, then hide each dec tail
                    # under the next group's encoder matmuls
                    nxt = [make_state(s) for s in groups[gi + 1]]
                    for st, nst in zip(sts, nxt):
                        dec_phase(st)
                        enc_phase(nst)
                    sts = nxt
                else:
                    for st in sts:
                        dec_phase(st)

    nc.compile()
    return nc


def host_inputs(x, W_enc, b_enc, W_h, b_h, W_dec, b_dec, n_cores=N_CORES, bs=BS):
    """Shard x over cores (pre-transposed to [DIN, tok]); replicate weights."""
    x = np.asarray(x, np.float32)
    common = {
        "w_enc": np.ascontiguousarray(np.asarray(W_enc, np.float32)),
        "b_enc": np.ascontiguousarray(
            np.asarray(b_enc, np.float32).reshape(2, 128).T
        ),
        "w_top": np.ascontiguousarray(
            np.asarray(W_h, np.float32)[:, :H, :].reshape(K, 2, 128, H)
        ),
        "w_bot": np.ascontiguousarray(
            (np.asarray(W_h, np.float32)[:, H:, :] / A).reshape(K, 2, 128, H)
        ),
        "b_h": np.ascontiguousarray(
            np.asarray(b_h, np.float32).reshape(K, 2, 128).transpose(2, 0, 1).reshape(128, K * 2)
        ),
        "w_dec": np.ascontiguousarray(
            np.asarray(W_dec, np.float32).reshape(2, 128, DOUT)
        ),
        "b_dec": np.ascontiguousarray(np.asarray(b_dec, np.float32).reshape(DOUT, 1)),
        "sel": np.ascontiguousarray(
            np.repeat(np.eye(ST_EX, dtype=np.float32), A, axis=1)
        ),
    }
    in_maps = []
    for i in range(n_cores):
        shard = x[i * bs : (i + 1) * bs].reshape(bs * A, DIN)
        in_maps.append({**common, "xT": np.ascontiguousarray(shard.T)})
    return in_maps


_NC_CACHE = None


def _get_nc():
    global _NC_CACHE
    if _NC_CACHE is None:
        _NC_CACHE = build_nc()
    return _NC_CACHE


def kernel(x, W_enc, b_enc, W_h, b_h, W_dec, b_dec, _run_kwargs=None):
    in_maps = host_inputs(x, W_enc, b_enc, W_h, b_h, W_dec, b_dec)
    nc = _get_nc()
    res = run_bass_kernel_spmd(nc, in_maps, list(range(N_CORES)), **(_run_kwargs or {}))
    outs = [
        res.results[i]["y"].T.reshape(BS, A, DOUT).astype(np.float32)
        for i in range(N_CORES)
    ]
    full = np.concatenate(outs, axis=0)
    if _run_kwargs:
        kernel.last_results = res
    return full
